# revision 40
# baseline (speedup 1.0000x reference)
"""Trainium2 Bass kernel for nn_Block_80736795230695 (AFNO block).

Math (see reference):
  n1 = LN1(x); attn = AFNO(n1) + n1; x2 = x + attn
  n2 = LN2(x2); h = n2 @ fc1 + b; h = dwconv3x3(h); h = gelu(h)
  out = x2 + h @ fc2 + b2

Decomposition (validated on host in numpy to ~1e-6 rel):
  - rfft2 is a separable DFT (two matmul stages + DMA relayout); irfft2
    is a dense matmul with Hermitian fold; mode packing m' as in
    host_derived.
  - LN1 gamma/beta fold into AFNO layer 1; LN2 gamma/beta fold into fc1.
  - fc1/fc2 run in fp8e4m3 with DoubleRow perf mode (weights scaled x16
    on host, compensated in the activation scale / output drain).
  - Depthwise 3x3 conv on a guard-padded 34x34 grid: 5 taps as diagonal
    matmuls on TensorE into a 3-bank PSUM tile (tap-outer order);
    center + 3 taps on VectorE accumulate in an f16 SBUF tile injected
    via an f16 identity matmul.
  - x input and y output are bf16 (halves the critical DMA windows);
    DMA issue order prioritizes x + DFT factors; large weight prefetch
    is issued after the DFT shuffle DMAs.

Sharding: data-parallel over batch: 16 batches -> 8 cores x 2.
"""

import numpy as np
import ml_dtypes

DIM = 384
NUM_BLOCKS = 8
BS = 48
HID = 1536
H = W = 32
WF = 17
NMODE = H * WF  # 544
LAM = 0.01
EPS = 1e-5
B_LOC = 2
N_CORES = 8

TAP_ACT = (0, 0)
TAPS_PE = [(-1, 0), (1, 0), (0, -1), (0, 1), (1, 1)]
TAPS_DVE = [(-1, -1), (-1, 1), (1, -1)]

PADW = 34
HP_LEN = 2 + PADW * PADW  # 1 guard + 34x34 + 1 guard = 1158
# conv output covers padded positions p in [34, 1122); psum idx = p - 34;
# h_pad read index for tap offset d is 1 + p + d (>= 0 because d >= -35).
CONV_LEN = 1088
CONV_HALF = 544
IDFT_CHUNKS = [128, 128, 128, 128, 32]


def _f32(a):
    return np.ascontiguousarray(np.asarray(a), dtype=np.float32)


def _bf16(a):
    return np.ascontiguousarray(np.asarray(a, np.float32).astype(ml_dtypes.bfloat16))


def _fp8(a):
    return np.ascontiguousarray(np.asarray(a, np.float32).astype(ml_dtypes.float8_e4m3))


FP8_WSCALE = 16.0
IDFT_SCALE = 16.0


def host_derived(inputs):
    ln1_g = np.asarray(inputs["ln1_g"], np.float64)
    ln1_b = np.asarray(inputs["ln1_b"], np.float64)
    ln2_g = np.asarray(inputs["ln2_g"], np.float64)
    ln2_b = np.asarray(inputs["ln2_b"], np.float64)
    w1 = np.asarray(inputs["afno_w1"], np.float64)
    b1 = np.asarray(inputs["afno_b1"], np.float64)
    w2 = np.asarray(inputs["afno_w2"], np.float64)
    b2 = np.asarray(inputs["afno_b2"], np.float64)
    fc1_w = np.asarray(inputs["fc1_w"], np.float64)
    fc1_b = np.asarray(inputs["fc1_b"], np.float64)
    dw_w = np.asarray(inputs["dw_w"], np.float64)
    dw_b = np.asarray(inputs["dw_b"], np.float64)
    fc2_w = np.asarray(inputs["fc2_w"], np.float64)
    fc2_b = np.asarray(inputs["fc2_b"], np.float64)

    # Separable forward DFT (ortho): per-stage scale 1/sqrt(32).
    # Mode order m' = 128*q + kh*4 + kw%4 for kw<16 (q=kw//4); 512+kh for kw=16.
    s1 = 1.0 / np.sqrt(32.0)
    # stage1 lhsT: [128 (h4, w), 80 (h4, kw', q)]  (q=4 cols with kw'>0 zero)
    fcos = np.zeros((128, 80))
    fsin = np.zeros((128, 80))
    for h4 in range(4):
        for w in range(32):
            row = h4 * 32 + w
            for kwp in range(4):
                for q in range(5):
                    if q == 4 and kwp > 0:
                        continue
                    kw = 16 if q == 4 else 4 * q + kwp
                    col = h4 * 20 + kwp * 5 + q
                    th = -2 * np.pi * kw * w / 32.0
                    fcos[row, col] = np.cos(th) * s1
                    fsin[row, col] = np.sin(th) * s1
    # stage2 rhs: [5][128 (h, kw'), 256 (ri, kh, kw')]
    fhr = np.zeros((5, 128, 256))
    fhi = np.zeros((5, 128, 256))
    for q in range(5):
        ncol = 128 if q < 4 else 32
        for h in range(32):
            for kwp in range(4):
                if q == 4 and kwp > 0:
                    continue
                row = 4 * h + kwp
                for kh in range(32):
                    col = kh * 4 + kwp if q < 4 else kh
                    phi = -2 * np.pi * kh * h / 32.0
                    fhr[q, row, col] = np.cos(phi) * s1
                    fhr[q, row, ncol + col] = np.sin(phi) * s1
                    fhi[q, row, col] = -np.sin(phi) * s1
                    fhi[q, row, ncol + col] = np.cos(phi) * s1

    # m' permutation (old mode m = kh*17 + kw -> m')
    mprime = np.zeros(NMODE, dtype=int)
    for kh in range(H):
        for kw in range(WF):
            if kw < 16:
                mprime[kh * WF + kw] = 128 * (kw // 4) + kh * 4 + (kw % 4)
            else:
                mprime[kh * WF + kw] = 512 + kh
    inv_mp = np.argsort(mprime)

    # inverse DFT with Hermitian fold, rows in m' order, chunked q = t*2 + r
    hh, ww = np.meshgrid(np.arange(H), np.arange(W), indexing="ij")
    khs = np.arange(H)[:, None, None, None]
    kws = np.arange(WF)[None, :, None, None]
    ang = -2 * np.pi * (khs * hh[None, None] / H + kws * ww[None, None] / W)
    sc = 1.0 / np.sqrt(H * W)
    d = np.where((np.arange(WF) == 0) | (np.arange(WF) == WF - 1), 1.0, 2.0)
    are = (np.cos(-ang) * d[None, :, None, None]) * sc
    aim = (-np.sin(-ang) * d[None, :, None, None]) * sc
    a_inv = np.concatenate(
        [are.reshape(NMODE, H * W)[inv_mp],
         aim.reshape(NMODE, H * W)[inv_mp]], axis=0)
    avq = np.zeros((5, 128, 2, H * W))
    row0 = 0
    for k, rows in enumerate(IDFT_CHUNKS):
        for r in range(2):
            avq[k, :rows, r, :] = (
                IDFT_SCALE * a_inv[r * NMODE + row0: r * NMODE + row0 + rows])
        row0 += rows

    # AFNO layer 1 with LN1 folds; zero-padded block-diagonal lhsT halves
    gblk = ln1_g.reshape(NUM_BLOCKS, BS, 1)
    w1r_f = w1[0] * gblk
    w1i_f = w1[1] * gblk
    dc_r = float(H) * np.einsum("ni,nio->no", ln1_b.reshape(NUM_BLOCKS, BS), w1[0])
    dc_i = float(H) * np.einsum("ni,nio->no", ln1_b.reshape(NUM_BLOCKS, BS), w1[1])
    w1c_a = np.zeros((96, NUM_BLOCKS * 96))
    w1c_b = np.zeros((96, NUM_BLOCKS * 96))
    b1c = np.zeros((96, NUM_BLOCKS))
    dcv = np.zeros((96, NUM_BLOCKS))
    for n in range(NUM_BLOCKS):
        r0 = (n % 2) * BS
        w1c_a[r0:r0 + BS, n * 96: n * 96 + BS] = w1r_f[n]
        w1c_a[r0:r0 + BS, n * 96 + BS: n * 96 + 96] = w1i_f[n]
        w1c_b[r0:r0 + BS, n * 96: n * 96 + BS] = -w1i_f[n]
        w1c_b[r0:r0 + BS, n * 96 + BS: n * 96 + 96] = w1r_f[n]
        b1c[:BS, n] = b1[0][n]
        b1c[BS:, n] = b1[1][n]
        dcv[:BS, n] = dc_r[n]
        dcv[BS:, n] = dc_i[n]

    # AFNO layer 2: lhsT is the o1 tile (rows 0:48 re, 48:96 im, 96 ones);
    # rhs [97, 96] per block, output cols (o, r) interleaved.
    w2c = np.zeros((97, NUM_BLOCKS * 96))
    for n in range(NUM_BLOCKS):
        blk = np.zeros((97, 96))
        blk[0:BS, 0:BS] = w2[0][n]
        blk[0:BS, BS:96] = w2[1][n]
        blk[BS:96, 0:BS] = -w2[1][n]
        blk[BS:96, BS:96] = w2[0][n]
        blk[96, 0:BS] = b2[0][n]
        blk[96, BS:96] = b2[1][n]
        w2c[:, n * 96: (n + 1) * 96] = blk

    w1f = fc1_w * ln2_g[:, None]
    b1f = fc1_b + ln2_b @ fc1_w

    dwk = dw_w[:, 0]  # [1536, 3, 3]

    def tapv(dy, dx):
        return dwk[:, dy + 1, dx + 1].reshape(12, 128).T

    dvev = np.stack([tapv(dy, dx) for (dy, dx) in TAPS_DVE],
                    axis=1)  # [128, 3, 12]
    ddiag = np.zeros((12, 128, len(TAPS_PE) * 128))
    for t in range(12):
        for j, (dy, dx) in enumerate(TAPS_PE):
            np.fill_diagonal(ddiag[t, :, j * 128: (j + 1) * 128],
                             dwk[t * 128: (t + 1) * 128, dy + 1, dx + 1])

    return {
        "fcos": _bf16(fcos),
        "fsin": _bf16(fsin),
        "fhr": _bf16(fhr),
        "fhi": _bf16(fhi),
        "avq": _fp8(avq),
        "w1ca": _bf16(w1c_a),
        "w1cb": _bf16(w1c_b),
        "b1c": _f32(b1c),
        "dcv": _f32(dcv),
        "w2c": _bf16(w2c),
        "ln1g": _bf16(np.broadcast_to(ln1_g[None, :], (128, DIM))),
        "ln1b": _bf16(np.broadcast_to(ln1_b[None, :], (128, DIM))),
        "w1fq": _fp8((FP8_WSCALE * w1f).reshape(3, 128, HID).transpose(1, 0, 2)),
        "b1fv": _f32(b1f.reshape(12, 128).T),
        "w2fq": _fp8((FP8_WSCALE * fc2_w).reshape(12, 128, DIM).transpose(1, 0, 2)),
        "b2f": _bf16(FP8_WSCALE * fc2_b.reshape(1, DIM)),
        "w0v": _f32(tapv(*TAP_ACT)),
        "dwbv": _f32(dw_b.reshape(12, 128).T),
        "dvev": _f32(dvev.reshape(128, 36)),  # col = j*12 + t
        "ddiag": _bf16(ddiag),
    }


def build_nc(ln1_trivial=True):
    import concourse.bass as bass
    import concourse.bacc as bacc
    import concourse.mybir as mybir
    import concourse.tile as tile
    from concourse.masks import make_identity
    from contextlib import ExitStack

    f32 = mybir.dt.float32
    bf16 = mybir.dt.bfloat16
    f8 = mybir.dt.float8e4
    DR = mybir.MatmulPerfMode.DoubleRow
    f16 = mybir.dt.float16
    ALU = mybir.AluOpType
    ACTF = mybir.ActivationFunctionType

    nc = bacc.Bacc("TRN2", target_bir_lowering=False, debug=False,
                   num_devices=N_CORES)

    xd = nc.dram_tensor("x", [B_LOC, 1024, DIM], bf16, kind="ExternalInput")
    yd = nc.dram_tensor("y", [B_LOC, 1024, DIM], bf16, kind="ExternalOutput")
    fcosd = nc.dram_tensor("fcos", [128, 80], bf16, kind="ExternalInput")
    fsind = nc.dram_tensor("fsin", [128, 80], bf16, kind="ExternalInput")
    fhrd = nc.dram_tensor("fhr", [5, 128, 256], bf16, kind="ExternalInput")
    fhid = nc.dram_tensor("fhi", [5, 128, 256], bf16, kind="ExternalInput")
    aid = nc.dram_tensor("avq", [5, 128, 2, 1024], f8, kind="ExternalInput")
    w1cad = nc.dram_tensor("w1ca", [96, 768], bf16, kind="ExternalInput")
    w1cbd = nc.dram_tensor("w1cb", [96, 768], bf16, kind="ExternalInput")
    b1cd = nc.dram_tensor("b1c", [96, 8], f32, kind="ExternalInput")
    dcvd = nc.dram_tensor("dcv", [96, 8], f32, kind="ExternalInput")
    w2cd = nc.dram_tensor("w2c", [97, 768], bf16, kind="ExternalInput")
    ln1gd = nc.dram_tensor("ln1g", [128, DIM], bf16, kind="ExternalInput")
    ln1bd = nc.dram_tensor("ln1b", [128, DIM], bf16, kind="ExternalInput")
    w1fd = nc.dram_tensor("w1fq", [128, 3, HID], f8, kind="ExternalInput")
    b1fvd = nc.dram_tensor("b1fv", [128, 12], f32, kind="ExternalInput")
    w2fd = nc.dram_tensor("w2fq", [128, 12, DIM], f8, kind="ExternalInput")
    b2fd = nc.dram_tensor("b2f", [1, DIM], bf16, kind="ExternalInput")
    w0vd = nc.dram_tensor("w0v", [128, 12], f32, kind="ExternalInput")
    dwbvd = nc.dram_tensor("dwbv", [128, 12], f32, kind="ExternalInput")
    dvevd = nc.dram_tensor("dvev", [128, 36], f32, kind="ExternalInput")
    ddiagd = nc.dram_tensor("ddiag", [12, 128, len(TAPS_PE) * 128], bf16,
                            kind="ExternalInput")

    def shifted(ap2d, elem_off, length):
        return bass.AP(tensor=ap2d.tensor, offset=ap2d.offset + elem_off,
                       ap=[ap2d.ap[0], [1, length]])

    with tile.TileContext(nc) as tc, \
         ExitStack() as ctx:
        dma = nc.sync.dma_start

        keep = ctx.enter_context(tc.tile_pool(name="keep", bufs=1))
        stat = ctx.enter_context(tc.tile_pool(name="stat", bufs=8))
        x2_pool = ctx.enter_context(tc.tile_pool(name="x2p", bufs=1))
        eps_t = keep.tile([128, 1], f32, tag="eps")
        nc.vector.memset(eps_t[:, :], EPS)
        ident = keep.tile([128, 128], bf16, tag="ident")
        make_identity(nc, ident[:, :])
        ident_h = keep.tile([128, 128], f16, tag="identh")
        make_identity(nc, ident_h[:, :])
        ident16 = keep.tile([128, 128], bf16, tag="ident16")
        make_identity(nc, ident16[:, :])
        nc.vector.tensor_scalar(out=ident16[:, :], in0=ident16[:, :],
                                scalar1=IDFT_SCALE, scalar2=None, op0=ALU.mult)


        # Persistent pools (never released) must sit below the transient
        # phase pools on the allocation stack.
        afw = ctx.enter_context(tc.tile_pool(name="afw", bufs=1))
        ai_pool = ctx.enter_context(tc.tile_pool(name="aip", bufs=1))
        mlpw = ctx.enter_context(tc.tile_pool(name="mlpw", bufs=1))

        # ---- DMA priority order: x first (gates LN1 -> DFT), then the DFT
        # factor matrices, then everything else (prefetch overlapping compute).
        x_pool = tc.alloc_tile_pool(name="xp", bufs=1)
        fw_pool = tc.alloc_tile_pool(name="fwp", bufs=1)
        xr = xd.ap().rearrange("b (n p) c -> n p b c", p=128)
        x_t = [x_pool.tile([128, B_LOC, DIM], bf16, tag=f"x{i}", name=f"x{i}")
               for i in range(8)]
        fcos_t = fw_pool.tile([128, 80], bf16, tag="fcos")
        fsin_t = fw_pool.tile([128, 80], bf16, tag="fsin")
        # first chunk + stage-1 factors first; odd chunks go out on the
        # scalar hwdge queue so both DMA rings pull concurrently.
        dma(out=x_t[0][:, :, :], in_=xr[0])
        dma(out=fcos_t[:, :], in_=fcosd.ap())
        dma(out=fsin_t[:, :], in_=fsind.ap())
        for i in range(1, 8):
            eng = dma if i % 2 == 0 else nc.scalar.dma_start
            eng(out=x_t[i][:, :, :], in_=xr[i])
        fhr_t, fhi_t = [], []
        for q in range(5):
            tr_ = fw_pool.tile([128, 256], bf16, tag=f"fhr{q}")
            ti_ = fw_pool.tile([128, 256], bf16, tag=f"fhi{q}")
            dma(out=tr_[:, :], in_=fhrd.ap()[q])
            dma(out=ti_[:, :], in_=fhid.ap()[q])
            fhr_t.append(tr_)
            fhi_t.append(ti_)

        w1ca = afw.tile([96, 768], bf16, tag="w1ca")
        w1cb = afw.tile([96, 768], bf16, tag="w1cb")
        w2c = afw.tile([97, 768], bf16, tag="w2c")
        b1c = afw.tile([96, 8], f32, tag="b1c")
        dcv = afw.tile([96, 8], f32, tag="dcv")
        dma(out=w1ca[:, :], in_=w1cad.ap())
        dma(out=w1cb[:, :], in_=w1cbd.ap())
        dma(out=w2c[:, :], in_=w2cd.ap())
        dma(out=b1c[:, :], in_=b1cd.ap())
        dma(out=dcv[:, :], in_=dcvd.ap())

        # Tiles for the later-phase weights (allocated now for pool-stack
        # order; their DMAs are issued after the DFT so they don't block the
        # DFT-internal shuffle DMAs on the queue).  hp memsets run on the
        # idle GpSimd engine during the startup DMA window.
        av = [ai_pool.tile([128, 2, 1024], f8, tag=f"av{q}", name=f"av{q}")
              for q in range(5)]
        w1fq = mlpw.tile([128, 3, HID], f8, tag="w1fq", name="w1fq")
        b1fv = mlpw.tile([128, 12], f32, tag="b1fv")
        w0v = mlpw.tile([128, 12], f32, tag="w0v")
        dwbv = mlpw.tile([128, 12], f32, tag="dwbv")
        dvev = mlpw.tile([128, 36], f32, tag="dvev")
        ddiag_t = [mlpw.tile([128, len(TAPS_PE) * 128], bf16, tag=f"dd{t}",
                             name=f"dd{t}") for t in range(12)]
        hp_t = []
        for t in range(12):
            hp = mlpw.tile([128, HP_LEN], bf16, tag=f"hp{t}")
            nc.gpsimd.memset(hp[:, :], 0.0)
            hp_t.append(hp)

        def layer_stats(xs, tagsuf):
            st = stat.tile([128, 6], f32, tag="st" + tagsuf)
            nc.vector.bn_stats(out=st[:, :], in_=xs)
            mv = stat.tile([128, 2], f32, tag="mv" + tagsuf)
            nc.vector.bn_aggr(out=mv[:, :], in_=st[:, :])
            rstd = stat.tile([128, 1], f32, tag="rs" + tagsuf)
            nc.scalar.activation(out=rstd[:, :], in_=mv[:, 1:2],
                                 func=ACTF.Sqrt, bias=eps_t[:, :])
            nc.vector.reciprocal(out=rstd[:, :], in_=rstd[:, :])
            return mv, rstd

        def layer_norm_act(out, xs, tagsuf, on_scalar=True):
            # normalize: out = xs*rstd + (-m*rstd); engine selectable so the
            # work can be split between ACT and DVE.
            mv, rstd = layer_stats(xs, tagsuf)
            if on_scalar:
                nmr = stat.tile([128, 1], f32, tag="nmr" + tagsuf)
                nc.vector.tensor_scalar(out=nmr[:, :], in0=mv[:, 0:1],
                                        scalar1=rstd[:, :], scalar2=-1.0,
                                        op0=ALU.mult, op1=ALU.mult)
                nc.scalar.activation(out=out, in_=xs, func=ACTF.Identity,
                                     scale=rstd[:, :], bias=nmr[:, :])
            else:
                nc.vector.tensor_scalar(out=out, in0=xs,
                                        scalar1=mv[:, 0:1],
                                        scalar2=rstd[:, :],
                                        op0=ALU.subtract, op1=ALU.mult)

        # ---------------- Phase A: LN1 (b-major) ----------------
        z1_pool = tc.alloc_tile_pool(name="z1p", bufs=1)
        n1_pool = tc.alloc_tile_pool(name="n1p", bufs=1)

        if not ln1_trivial:
            g_bc = n1_pool.tile([128, DIM], bf16, tag="g_bc")
            b_bc = n1_pool.tile([128, DIM], bf16, tag="b_bc")
            dma(out=g_bc[:, :], in_=ln1gd.ap())
            dma(out=b_bc[:, :], in_=ln1bd.ap())

        z1_t, n1_t = [], []
        with nc.named_scope("ln1"):
            for i in range(8):
                z1 = z1_pool.tile([128, B_LOC, DIM], bf16, tag=f"z1_{i}")
                z1_t.append(z1)
                if ln1_trivial:
                    n1_t.append(z1)  # n1 == z1 when gamma=1, beta=0
                else:
                    n1_t.append(n1_pool.tile([128, B_LOC, DIM], bf16,
                                             tag=f"n1_{i}"))
            for b in range(B_LOC):
                for i in range(8):
                    xs = x_t[i][:, b, :]
                    layer_norm_act(z1_t[i][:, b, :], xs, "1",
                                   on_scalar=(i % 2 == 0))
                    if not ln1_trivial:
                        nc.vector.tensor_mul(out=n1_t[i][:, b, :],
                                             in0=z1_t[i][:, b, :],
                                             in1=g_bc[:, :])
                        nc.vector.tensor_add(out=n1_t[i][:, b, :],
                                             in0=n1_t[i][:, b, :],
                                             in1=b_bc[:, :])

        # ---------------- Phase B: separable forward DFT ----------------
        xf_pool = tc.alloc_tile_pool(name="xfp", bufs=1)
        xf_t = [xf_pool.tile([96, 2 * NMODE], bf16, tag=f"xf{mt}",
                             name=f"xf{mt}") for mt in range(8)]
        with nc.named_scope("dft"):
            with tc.tile_pool(name="xwp", bufs=4) as xw_pool, \
                 tc.tile_pool(name="xhp", bufs=1) as xh_pool, \
                 tc.tile_pool(name="s1ps", bufs=4, space="PSUM") as s1_ps, \
                 tc.tile_pool(name="s2ps", bufs=4, space="PSUM") as s2_ps:
                # xh: [128 (h, kw'), 5 (q), B, DIM]
                xh_r = xh_pool.tile([128, 5, B_LOC, DIM], bf16, tag="xhr")
                xh_i = xh_pool.tile([128, 5, B_LOC, DIM], bf16, tag="xhi")
                for b in range(B_LOC):
                    # stage 1: W-direction rDFT per position chunk
                    for i in range(8):
                        psr = s1_ps.tile([80, DIM], f32, tag="s1ps")
                        psi = s1_ps.tile([80, DIM], f32, tag="s1ps")
                        nc.tensor.matmul(psr[:, :], fcos_t[:, :],
                                         z1_t[i][:, b, :],
                                         start=True, stop=True)
                        nc.tensor.matmul(psi[:, :], fsin_t[:, :],
                                         z1_t[i][:, b, :],
                                         start=True, stop=True)
                        xwr = xw_pool.tile([80, DIM], bf16, tag="xwr")
                        xwi = xw_pool.tile([80, DIM], bf16, tag="xwi")
                        nc.scalar.copy(out=xwr[:, :], in_=psr[:, :])
                        nc.vector.tensor_copy(out=xwi[:, :], in_=psi[:, :])
                        # (h4, kw', q) partitions -> (h, kw') x (q) free
                        dma(out=xh_r[16 * i: 16 * (i + 1), :, b, :],
                            in_=xwr[:, :])
                        dma(out=xh_i[16 * i: 16 * (i + 1), :, b, :],
                            in_=xwi[:, :])
                    # stage 2: H-direction DFT; lhsT = data, rhs = factors
                    for q in range(5):
                        ncol = 128 if q < 4 else 32
                        for c96 in range(4):
                            ps2 = s2_ps.tile([96, 256], f32, tag="s2ps")
                            nc.tensor.matmul(
                                ps2[:, : 2 * ncol],
                                xh_r[:, q, b, 96 * c96: 96 * (c96 + 1)],
                                fhr_t[q][:, : 2 * ncol],
                                start=True, stop=False)
                            nc.tensor.matmul(
                                ps2[:, : 2 * ncol],
                                xh_i[:, q, b, 96 * c96: 96 * (c96 + 1)],
                                fhi_t[q][:, : 2 * ncol],
                                start=False, stop=True)
                            xf2d = xf_t[b * 4 + c96][:, :]
                            dest = bass.AP(
                                tensor=xf2d.tensor,
                                offset=xf2d.offset + 128 * q,
                                ap=[xf2d.ap[0], [NMODE, 2], [1, ncol]])
                            cp_eng = (nc.scalar.copy if c96 % 2 == 0
                                      else nc.vector.tensor_copy)
                            cp_eng(
                                out=dest,
                                in_=ps2[:, : 2 * ncol]
                                    .rearrange("p (r m) -> p r m", r=2))

        # ---- weight prefetch: issued here so these queue BEHIND the DFT
        # shuffle DMAs but still land well before the idft / MLP need them.
        for q in range(5):
            dma(out=av[q][:, :, :], in_=aid.ap()[q])
        dma(out=w1fq[:, :, :], in_=w1fd.ap())
        dma(out=b1fv[:, :], in_=b1fvd.ap())
        dma(out=w0v[:, :], in_=w0vd.ap())
        dma(out=dwbv[:, :], in_=dwbvd.ap())
        dma(out=dvev[:, :], in_=dvevd.ap())
        for t in range(12):
            dma(out=ddiag_t[t][:, :], in_=ddiagd.ap()[t])

        # ---------------- Phase C: frequency-domain block MLP ----------------
        o2s_pool = tc.alloc_tile_pool(name="o2sp", bufs=1)
        o2s_t = [o2s_pool.tile([IDFT_CHUNKS[t], B_LOC, 2, DIM], f8,
                               tag=f"o2s{t}", name=f"o2s{t}") for t in range(5)]
        with nc.named_scope("blockmlp"):
            with tc.tile_pool(name="o1p", bufs=8) as o1_pool, \
                 tc.tile_pool(name="l1ps", bufs=3, space="PSUM") as l1_ps, \
                 tc.tile_pool(name="l2ps", bufs=2, space="PSUM") as l2_ps:
                for b in range(B_LOC):
                    o1_b = []
                    for n in range(NUM_BLOCKS):
                        xf = xf_t[b * 4 + n // 2]
                        o1 = o1_pool.tile([97, NMODE], bf16, tag="o1")
                        nc.gpsimd.memset(o1[96:97, :], 1.0)
                        ps = l1_ps.tile([96, 544], f32, tag="l1ps")
                        for w0, wlen in ((0, 512), (512, 32)):
                            nc.tensor.matmul(
                                ps[:, w0: w0 + wlen],
                                w1ca[:, 96 * n: 96 * (n + 1)],
                                xf[:, w0: w0 + wlen],
                                start=True, stop=False)
                            nc.tensor.matmul(
                                ps[:, w0: w0 + wlen],
                                w1cb[:, 96 * n: 96 * (n + 1)],
                                xf[:, NMODE + w0: NMODE + w0 + wlen],
                                start=False, stop=True)
                        if not ln1_trivial:
                            # DC-mode correction from LN1 beta (zero if trivial)
                            nc.vector.tensor_add(out=ps[:, 0:1], in0=ps[:, 0:1],
                                                 in1=dcv[:, n: n + 1])
                        nc.scalar.activation(out=o1[0:96, :], in_=ps[:, :],
                                             func=ACTF.Relu,
                                             bias=b1c[:, n: n + 1])
                        o1_b.append(o1)
                    for mc in range(5):
                        rows = IDFT_CHUNKS[mc]
                        for hn in range(2):
                            ps2 = l2_ps.tile([128, 384], f32, tag="l2ps")
                            for j in range(4):
                                n = hn * 4 + j
                                nc.tensor.matmul(
                                    ps2[:rows, 96 * j: 96 * (j + 1)],
                                    o1_b[n][:, 128 * mc: 128 * mc + rows],
                                    w2c[:, 96 * n: 96 * (n + 1)],
                                    start=True, stop=True)
                            cp2 = (nc.scalar.copy if (mc + hn) % 2 == 0
                                   else nc.vector.tensor_copy)
                            o2v = o2s_t[mc][:rows, b, :, :]
                            dest = bass.AP(
                                tensor=o2v.tensor,
                                offset=o2v.offset + 192 * hn,
                                ap=[o2v.ap[0], [BS, 4], [DIM, 2],
                                    [1, BS]])
                            cp2(
                                out=dest,
                                in_=ps2[:rows, :]
                                    .rearrange("p (j r o) -> p j r o",
                                               j=4, r=2))

        # softshrink in place
        with nc.named_scope("softshrink"):
            with tc.tile_pool(name="sshp", bufs=2) as ssh_pool:
                for t in range(5):
                    rows = IDFT_CHUNKS[t]
                    flat = o2s_t[t][:, :, :, :].rearrange("p b r c -> p (b r c)")
                    tmp = ssh_pool.tile([128, B_LOC * DIM * 2], bf16, tag="ssh")
                    nc.vector.tensor_scalar(out=tmp[:rows, :], in0=flat,
                                            scalar1=-LAM, scalar2=LAM,
                                            op0=ALU.max, op1=ALU.min)
                    nc.vector.tensor_tensor(
                        out=flat, in0=flat, in1=tmp[:rows, :],
                        op=ALU.subtract)

        # ---------------- Phase D: inverse DFT + residual ----------------
        x2_t = []
        with nc.named_scope("idft"):
            with tc.tile_pool(name="idftps", bufs=4, space="PSUM") as idft_ps:
                for mt in range(8):
                    x2 = x2_pool.tile([128, B_LOC, DIM], bf16, tag=f"x2_{mt}")
                    for b in range(B_LOC):
                        ps = idft_ps.tile([128, DIM], f32, tag="idftps")
                        for q in range(5):
                            rows = IDFT_CHUNKS[q]
                            nc.tensor.matmul(
                                ps[:, :],
                                av[q][:rows, :, 128 * mt: 128 * (mt + 1)],
                                o2s_t[q][:rows, b, :, :],
                                start=(q == 0), stop=False,
                                perf_mode=DR)
                        # + n1 residual (z1 when LN1 is trivial), x16 to
                        # match the scaled idft factors
                        nc.tensor.matmul(
                            ps[:, :], ident16[:, :], n1_t[mt][:, b, :],
                            start=False, stop=True)
                        nc.vector.scalar_tensor_tensor(
                            out=x2[:, b, :], in0=ps[:, :],
                            scalar=1.0 / IDFT_SCALE,
                            in1=x_t[mt][:, b, :],
                            op0=ALU.mult, op1=ALU.add)
                    x2_t.append(x2)
        o2s_pool.release()
        xf_pool.release()
        n1_pool.release()
        z1_pool.release()
        fw_pool.release()
        x_pool.release()

        # ---------------- Phase E: MLP (late weights) ----------------
        mlpw2 = ctx.enter_context(tc.tile_pool(name="mlpw2", bufs=1))
        w2fq = mlpw2.tile([128, 12, DIM], f8, tag="w2fq", name="w2fq")
        dma(out=w2fq[:, :, :], in_=w2fd.ap())
        b2f = mlpw2.tile([1, DIM], bf16, tag="b2f")
        dma(out=b2f[:, :], in_=b2fd.ap())
        ones_b = mlpw2.tile([1, 1024], bf16, tag="ones")
        nc.vector.memset(ones_b[:, :], 1.0)
        yq = mlpw2.tile([128, 12, 1024], f8, tag="yq", name="yq")
        # z2T as one [128, 3 (ch-chunk), 1024 (pos)] tile
        z2T = mlpw2.tile([128, 3, 1024], f8, tag="z2T", name="z2T")

        with tc.tile_pool(name="z2p", bufs=3) as z2_pool, \
             tc.tile_pool(name="accp", bufs=3) as acc_pool, \
             tc.tile_pool(name="outp", bufs=3) as out_pool, \
             tc.tile_pool(name="mlpps", bufs=2, space="PSUM") as mlp_ps, \
             tc.tile_pool(name="cvps", bufs=2, space="PSUM") as cv_ps:
            for b in range(B_LOC):
                with nc.named_scope("ln2t"):
                    for i in range(8):
                        xs = x2_t[i][:, b, :]
                        z2 = z2_pool.tile([128, DIM], f16, tag="z2")
                        layer_norm_act(z2[:, :], xs, "2",
                                       on_scalar=(i % 2 == 0))
                        pstf = mlp_ps.tile([128, 512], f32, tag="fc1ps")
                        pst = pstf[:, :].bitcast(f16)
                        for c in range(3):
                            nc.tensor.transpose(
                                pst[:, 128 * c: 128 * (c + 1)],
                                z2[:, 128 * c: 128 * (c + 1)],
                                ident_h[:, :])
                        nc.scalar.copy(
                            out=z2T[:, :, 128 * i: 128 * (i + 1)],
                            in_=pst[:, :384].rearrange("p (c m) -> p c m", c=3))
                with nc.named_scope("fc1"):
                    for t in range(12):
                        for ncb in range(2):
                            ps = mlp_ps.tile([128, 512], f32, tag="fc1ps")
                            nc.tensor.matmul(
                                ps[:, :],
                                w1fq[:, 0:2, 128 * t: 128 * (t + 1)],
                                z2T[:, 0:2, 512 * ncb: 512 * (ncb + 1)],
                                start=True, stop=False, perf_mode=DR)
                            nc.tensor.matmul(
                                ps[:, :],
                                w1fq[:, 2, 128 * t: 128 * (t + 1)],
                                z2T[:, 2, 512 * ncb: 512 * (ncb + 1)],
                                start=False, stop=True)
                            hp2d = hp_t[t][:, :]
                            dest = bass.AP(
                                tensor=hp2d.tensor,
                                offset=hp2d.offset + 36 + 544 * ncb,
                                ap=[hp2d.ap[0], [34, 16], [1, 32]])
                            nc.scalar.activation(
                                out=dest,
                                in_=ps[:, :].rearrange("p (h w) -> p h w", w=32),
                                func=ACTF.Identity, bias=b1fv[:, t: t + 1],
                                scale=1.0 / FP8_WSCALE)
                with nc.named_scope("conv"):
                    for t in range(12):
                        hp2d = hp_t[t][:, :]
                        # fp16 SBUF accumulator: center + 3 taps on DVE.
                        # Each tap is tensor_scalar (4x mode) + tensor_tensor
                        # (2x mode) -- cheaper than 1x scalar_tensor_tensor.
                        acc = acc_pool.tile([128, CONV_LEN], f16, tag="acc")
                        nc.vector.tensor_scalar(
                            out=acc[:, :], in0=shifted(hp2d, 35, CONV_LEN),
                            scalar1=w0v[:, t: t + 1],
                            scalar2=dwbv[:, t: t + 1],
                            op0=ALU.mult, op1=ALU.add)
                        for j, (dy, dx) in enumerate(TAPS_DVE):
                            dd = dy * PADW + dx
                            tmp = acc_pool.tile([128, CONV_LEN], bf16,
                                                tag="tmpt")
                            nc.vector.tensor_scalar(
                                out=tmp[:, :],
                                in0=shifted(hp2d, 35 + dd, CONV_LEN),
                                scalar1=dvev[:, 12 * j + t: 12 * j + t + 1],
                                scalar2=None,
                                op0=ALU.mult)
                            nc.vector.tensor_tensor(
                                out=acc[:, :], in0=tmp[:, :], in1=acc[:, :],
                                op=ALU.add)
                        # PE taps: one 3-bank PSUM tile; tap-outer order so
                        # consecutive matmuls reuse the loaded weights.
                        cps = cv_ps.tile([128, CONV_LEN], f32, tag="cvps")
                        for j, (dy, dx) in enumerate(TAPS_PE):
                            dd = dy * PADW + dx
                            for w0, wlen in ((0, 512), (512, 512), (1024, 64)):
                                nc.tensor.matmul(
                                    cps[:, w0: w0 + wlen],
                                    ddiag_t[t][:, 128 * j: 128 * (j + 1)],
                                    shifted(hp2d, 35 + w0 + dd, wlen),
                                    start=(j == 0), stop=False,
                                    skip_group_check=True)
                        # inject the DVE accumulator via identity matmul
                        for w0, wlen in ((0, 512), (512, 512), (1024, 64)):
                            nc.tensor.matmul(
                                cps[:, w0: w0 + wlen],
                                ident_h[:, :],
                                shifted(acc[:, :], w0, wlen),
                                start=False, stop=True,
                                skip_group_check=True)
                        cps2d = cps[:, :]
                        for hf in range(2):
                            nc.scalar.activation(
                                out=yq[:, t, 512 * hf: 512 * (hf + 1)]
                                    .rearrange("p (h w) -> p h w", w=32),
                                in_=bass.AP(tensor=cps2d.tensor,
                                            offset=cps2d.offset + 1
                                            + CONV_HALF * hf,
                                            ap=[cps2d.ap[0], [34, 16], [1, 32]]),
                                func=ACTF.Gelu)
                with nc.named_scope("fc2"):
                    for mc in range(8):
                        psf = mlp_ps.tile([128, 512], f32, tag="fc1ps")
                        ps = psf[:, :DIM]
                        for j in range(6):
                            nc.tensor.matmul(
                                ps[:, :],
                                yq[:, 2 * j: 2 * j + 2, 128 * mc: 128 * (mc + 1)],
                                w2fq[:, 2 * j: 2 * j + 2, :],
                                start=(j == 0), stop=False, perf_mode=DR)
                        nc.tensor.matmul(
                            ps[:, :], ones_b[:, 128 * mc: 128 * (mc + 1)],
                            b2f[:, :], start=False, stop=True)
                        # 1/16 weight-scale compensation + x2 residual, on DVE
                        ot = out_pool.tile([128, DIM], bf16, tag="out")
                        nc.vector.scalar_tensor_tensor(
                            out=ot[:, :], in0=ps[:, :], scalar=1.0 / FP8_WSCALE,
                            in1=x2_t[mc][:, b, :], op0=ALU.mult, op1=ALU.add)
                        dma(out=yd.ap()[b, 128 * mc: 128 * (mc + 1), :],
                            in_=ot[:, :])

    return nc


_NC_CACHE = {}


def kernel(**inputs):
    from concourse.bass_utils import run_bass_kernel_spmd

    x = np.ascontiguousarray(np.asarray(inputs["x"], np.float32))
    assert int(inputs["H"]) == H and int(inputs["W"]) == W
    der = host_derived(inputs)
    trivial = (np.allclose(np.asarray(inputs["ln1_g"]), 1.0)
               and np.allclose(np.asarray(inputs["ln1_b"]), 0.0))

    if trivial not in _NC_CACHE:
        nc = build_nc(ln1_trivial=trivial)
        nc.compile()
        _NC_CACHE[trivial] = nc
    nc = _NC_CACHE[trivial]

    x_bf = x.astype(ml_dtypes.bfloat16)
    in_maps = []
    for c in range(N_CORES):
        m = dict(der)
        m["x"] = np.ascontiguousarray(x_bf[c * B_LOC: (c + 1) * B_LOC])
        in_maps.append(m)
    res = run_bass_kernel_spmd(nc, in_maps, core_ids=list(range(N_CORES)))
    out = np.concatenate([res.results[c]["y"] for c in range(N_CORES)], axis=0)
    return out.astype(np.float32)



# revision 41
# speedup vs baseline: 1.0226x; 1.0226x over previous
"""Trainium2 Bass kernel for nn_Block_80736795230695 (AFNO block).

Math (see reference):
  n1 = LN1(x); attn = AFNO(n1) + n1; x2 = x + attn
  n2 = LN2(x2); h = n2 @ fc1 + b; h = dwconv3x3(h); h = gelu(h)
  out = x2 + h @ fc2 + b2

Decomposition (validated on host in numpy to ~1e-6 rel):
  - rfft2 is a separable DFT (two matmul stages + DMA relayout); irfft2
    is a dense matmul with Hermitian fold; mode packing m' as in
    host_derived.
  - LN1 gamma/beta fold into AFNO layer 1; LN2 gamma/beta fold into fc1.
  - fc1/fc2 run in fp8e4m3 with DoubleRow perf mode (weights scaled x16
    on host, compensated in the activation scale / output drain).
  - Depthwise 3x3 conv on a guard-padded 34x34 grid: 5 taps as diagonal
    matmuls on TensorE into a 3-bank PSUM tile (tap-outer order);
    center + 3 taps on VectorE accumulate in an f16 SBUF tile injected
    via an f16 identity matmul.
  - x input and y output are bf16 (halves the critical DMA windows);
    DMA issue order prioritizes x + DFT factors; large weight prefetch
    is issued after the DFT shuffle DMAs.

Sharding: data-parallel over batch: 16 batches -> 8 cores x 2.
"""

import numpy as np
import ml_dtypes

DIM = 384
NUM_BLOCKS = 8
BS = 48
HID = 1536
H = W = 32
WF = 17
NMODE = H * WF  # 544
LAM = 0.01
EPS = 1e-5
B_LOC = 2
N_CORES = 8

TAP_ACT = (0, 0)
TAPS_PE = [(-1, 0), (1, 0), (0, -1), (0, 1), (1, 1)]
TAPS_DVE = [(-1, -1), (-1, 1), (1, -1)]

PADW = 34
HP_LEN = 2 + PADW * PADW  # 1 guard + 34x34 + 1 guard = 1158
# conv output covers padded positions p in [34, 1122); psum idx = p - 34;
# h_pad read index for tap offset d is 1 + p + d (>= 0 because d >= -35).
CONV_LEN = 1088
CONV_HALF = 544
IDFT_CHUNKS = [128, 128, 128, 128, 32]


def _f32(a):
    return np.ascontiguousarray(np.asarray(a), dtype=np.float32)


def _bf16(a):
    return np.ascontiguousarray(np.asarray(a, np.float32).astype(ml_dtypes.bfloat16))


def _fp8(a):
    return np.ascontiguousarray(np.asarray(a, np.float32).astype(ml_dtypes.float8_e4m3))


FP8_WSCALE = 16.0
IDFT_SCALE = 16.0


def host_derived(inputs):
    ln1_g = np.asarray(inputs["ln1_g"], np.float64)
    ln1_b = np.asarray(inputs["ln1_b"], np.float64)
    ln2_g = np.asarray(inputs["ln2_g"], np.float64)
    ln2_b = np.asarray(inputs["ln2_b"], np.float64)
    w1 = np.asarray(inputs["afno_w1"], np.float64)
    b1 = np.asarray(inputs["afno_b1"], np.float64)
    w2 = np.asarray(inputs["afno_w2"], np.float64)
    b2 = np.asarray(inputs["afno_b2"], np.float64)
    fc1_w = np.asarray(inputs["fc1_w"], np.float64)
    fc1_b = np.asarray(inputs["fc1_b"], np.float64)
    dw_w = np.asarray(inputs["dw_w"], np.float64)
    dw_b = np.asarray(inputs["dw_b"], np.float64)
    fc2_w = np.asarray(inputs["fc2_w"], np.float64)
    fc2_b = np.asarray(inputs["fc2_b"], np.float64)

    # Separable forward DFT (ortho): per-stage scale 1/sqrt(32).
    # Mode order m' = 128*q + kh*4 + kw%4 for kw<16 (q=kw//4); 512+kh for kw=16.
    s1 = 1.0 / np.sqrt(32.0)
    # stage1 lhsT: [128 (h4, w), 80 (h4, kw', q)]  (q=4 cols with kw'>0 zero)
    fcos = np.zeros((128, 80))
    fsin = np.zeros((128, 80))
    for h4 in range(4):
        for w in range(32):
            row = h4 * 32 + w
            for kwp in range(4):
                for q in range(5):
                    if q == 4 and kwp > 0:
                        continue
                    kw = 16 if q == 4 else 4 * q + kwp
                    col = h4 * 20 + kwp * 5 + q
                    th = -2 * np.pi * kw * w / 32.0
                    fcos[row, col] = np.cos(th) * s1
                    fsin[row, col] = np.sin(th) * s1
    # stage2 rhs: [5][128 (h, kw'), 256 (ri, kh, kw')]
    fhr = np.zeros((5, 128, 256))
    fhi = np.zeros((5, 128, 256))
    for q in range(5):
        ncol = 128 if q < 4 else 32
        for h in range(32):
            for kwp in range(4):
                if q == 4 and kwp > 0:
                    continue
                row = 4 * h + kwp
                for kh in range(32):
                    col = kh * 4 + kwp if q < 4 else kh
                    phi = -2 * np.pi * kh * h / 32.0
                    fhr[q, row, col] = np.cos(phi) * s1
                    fhr[q, row, ncol + col] = np.sin(phi) * s1
                    fhi[q, row, col] = -np.sin(phi) * s1
                    fhi[q, row, ncol + col] = np.cos(phi) * s1

    # m' permutation (old mode m = kh*17 + kw -> m')
    mprime = np.zeros(NMODE, dtype=int)
    for kh in range(H):
        for kw in range(WF):
            if kw < 16:
                mprime[kh * WF + kw] = 128 * (kw // 4) + kh * 4 + (kw % 4)
            else:
                mprime[kh * WF + kw] = 512 + kh
    inv_mp = np.argsort(mprime)

    # inverse DFT with Hermitian fold, rows in m' order, chunked q = t*2 + r
    hh, ww = np.meshgrid(np.arange(H), np.arange(W), indexing="ij")
    khs = np.arange(H)[:, None, None, None]
    kws = np.arange(WF)[None, :, None, None]
    ang = -2 * np.pi * (khs * hh[None, None] / H + kws * ww[None, None] / W)
    sc = 1.0 / np.sqrt(H * W)
    d = np.where((np.arange(WF) == 0) | (np.arange(WF) == WF - 1), 1.0, 2.0)
    are = (np.cos(-ang) * d[None, :, None, None]) * sc
    aim = (-np.sin(-ang) * d[None, :, None, None]) * sc
    a_inv = np.concatenate(
        [are.reshape(NMODE, H * W)[inv_mp],
         aim.reshape(NMODE, H * W)[inv_mp]], axis=0)
    a_ord = np.zeros_like(a_inv)
    off = 0
    row0 = 0
    for rows in IDFT_CHUNKS:
        for r in range(2):
            a_ord[off:off + rows] = a_inv[r * NMODE + row0: r * NMODE + row0 + rows]
            off += rows
        row0 += rows

    # AFNO layer 1 with LN1 folds; zero-padded block-diagonal lhsT halves
    gblk = ln1_g.reshape(NUM_BLOCKS, BS, 1)
    w1r_f = w1[0] * gblk
    w1i_f = w1[1] * gblk
    dc_r = float(H) * np.einsum("ni,nio->no", ln1_b.reshape(NUM_BLOCKS, BS), w1[0])
    dc_i = float(H) * np.einsum("ni,nio->no", ln1_b.reshape(NUM_BLOCKS, BS), w1[1])
    w1c_a = np.zeros((96, NUM_BLOCKS * 96))
    w1c_b = np.zeros((96, NUM_BLOCKS * 96))
    b1c = np.zeros((96, NUM_BLOCKS))
    dcv = np.zeros((96, NUM_BLOCKS))
    for n in range(NUM_BLOCKS):
        r0 = (n % 2) * BS
        w1c_a[r0:r0 + BS, n * 96: n * 96 + BS] = w1r_f[n]
        w1c_a[r0:r0 + BS, n * 96 + BS: n * 96 + 96] = w1i_f[n]
        w1c_b[r0:r0 + BS, n * 96: n * 96 + BS] = -w1i_f[n]
        w1c_b[r0:r0 + BS, n * 96 + BS: n * 96 + 96] = w1r_f[n]
        b1c[:BS, n] = b1[0][n]
        b1c[BS:, n] = b1[1][n]
        dcv[:BS, n] = dc_r[n]
        dcv[BS:, n] = dc_i[n]

    # AFNO layer 2: lhsT is the o1 tile (rows 0:48 re, 48:96 im, 96 ones);
    # rhs [97, 96] per block, output cols (o, r) interleaved.
    w2c = np.zeros((97, NUM_BLOCKS * 96))
    for n in range(NUM_BLOCKS):
        blk = np.zeros((97, 96))
        blk[0:BS, 0::2] = w2[0][n]
        blk[0:BS, 1::2] = w2[1][n]
        blk[BS:96, 0::2] = -w2[1][n]
        blk[BS:96, 1::2] = w2[0][n]
        blk[96, 0::2] = b2[0][n]
        blk[96, 1::2] = b2[1][n]
        w2c[:, n * 96: (n + 1) * 96] = blk

    w1f = fc1_w * ln2_g[:, None]
    b1f = fc1_b + ln2_b @ fc1_w

    dwk = dw_w[:, 0]  # [1536, 3, 3]

    def tapv(dy, dx):
        return dwk[:, dy + 1, dx + 1].reshape(12, 128).T

    dvev = np.stack([tapv(dy, dx) for (dy, dx) in TAPS_DVE],
                    axis=1)  # [128, 3, 12]
    ddiag = np.zeros((12, 128, len(TAPS_PE) * 128))
    for t in range(12):
        for j, (dy, dx) in enumerate(TAPS_PE):
            np.fill_diagonal(ddiag[t, :, j * 128: (j + 1) * 128],
                             dwk[t * 128: (t + 1) * 128, dy + 1, dx + 1])

    return {
        "fcos": _bf16(fcos),
        "fsin": _bf16(fsin),
        "fhr": _bf16(fhr),
        "fhi": _bf16(fhi),
        "ainv": _bf16(a_ord),
        "w1ca": _bf16(w1c_a),
        "w1cb": _bf16(w1c_b),
        "b1c": _f32(b1c),
        "dcv": _f32(dcv),
        "w2c": _bf16(w2c),
        "ln1g": _bf16(np.broadcast_to(ln1_g[None, :], (128, DIM))),
        "ln1b": _bf16(np.broadcast_to(ln1_b[None, :], (128, DIM))),
        "w1fq": _fp8((FP8_WSCALE * w1f).reshape(3, 128, HID).transpose(1, 0, 2)),
        "b1fv": _f32(b1f.reshape(12, 128).T),
        "w2fq": _fp8((FP8_WSCALE * fc2_w).reshape(12, 128, DIM).transpose(1, 0, 2)),
        "b2f": _bf16(FP8_WSCALE * fc2_b.reshape(1, DIM)),
        "w0v": _f32(tapv(*TAP_ACT)),
        "dwbv": _f32(dw_b.reshape(12, 128).T),
        "dvev": _f32(dvev.reshape(128, 36)),  # col = j*12 + t
        "ddiag": _bf16(ddiag),
    }


def build_nc(ln1_trivial=True):
    import concourse.bass as bass
    import concourse.bacc as bacc
    import concourse.mybir as mybir
    import concourse.tile as tile
    from concourse.masks import make_identity
    from contextlib import ExitStack

    f32 = mybir.dt.float32
    bf16 = mybir.dt.bfloat16
    f8 = mybir.dt.float8e4
    DR = mybir.MatmulPerfMode.DoubleRow
    f16 = mybir.dt.float16
    ALU = mybir.AluOpType
    ACTF = mybir.ActivationFunctionType

    nc = bacc.Bacc("TRN2", target_bir_lowering=False, debug=False,
                   num_devices=N_CORES)

    xd = nc.dram_tensor("x", [B_LOC, 1024, DIM], bf16, kind="ExternalInput")
    yd = nc.dram_tensor("y", [B_LOC, 1024, DIM], bf16, kind="ExternalOutput")
    fcosd = nc.dram_tensor("fcos", [128, 80], bf16, kind="ExternalInput")
    fsind = nc.dram_tensor("fsin", [128, 80], bf16, kind="ExternalInput")
    fhrd = nc.dram_tensor("fhr", [5, 128, 256], bf16, kind="ExternalInput")
    fhid = nc.dram_tensor("fhi", [5, 128, 256], bf16, kind="ExternalInput")
    aid = nc.dram_tensor("ainv", [2 * NMODE, 1024], bf16, kind="ExternalInput")
    w1cad = nc.dram_tensor("w1ca", [96, 768], bf16, kind="ExternalInput")
    w1cbd = nc.dram_tensor("w1cb", [96, 768], bf16, kind="ExternalInput")
    b1cd = nc.dram_tensor("b1c", [96, 8], f32, kind="ExternalInput")
    dcvd = nc.dram_tensor("dcv", [96, 8], f32, kind="ExternalInput")
    w2cd = nc.dram_tensor("w2c", [97, 768], bf16, kind="ExternalInput")
    ln1gd = nc.dram_tensor("ln1g", [128, DIM], bf16, kind="ExternalInput")
    ln1bd = nc.dram_tensor("ln1b", [128, DIM], bf16, kind="ExternalInput")
    w1fd = nc.dram_tensor("w1fq", [128, 3, HID], f8, kind="ExternalInput")
    b1fvd = nc.dram_tensor("b1fv", [128, 12], f32, kind="ExternalInput")
    w2fd = nc.dram_tensor("w2fq", [128, 12, DIM], f8, kind="ExternalInput")
    b2fd = nc.dram_tensor("b2f", [1, DIM], bf16, kind="ExternalInput")
    w0vd = nc.dram_tensor("w0v", [128, 12], f32, kind="ExternalInput")
    dwbvd = nc.dram_tensor("dwbv", [128, 12], f32, kind="ExternalInput")
    dvevd = nc.dram_tensor("dvev", [128, 36], f32, kind="ExternalInput")
    ddiagd = nc.dram_tensor("ddiag", [12, 128, len(TAPS_PE) * 128], bf16,
                            kind="ExternalInput")

    def shifted(ap2d, elem_off, length):
        return bass.AP(tensor=ap2d.tensor, offset=ap2d.offset + elem_off,
                       ap=[ap2d.ap[0], [1, length]])

    with tile.TileContext(nc) as tc, \
         ExitStack() as ctx:
        dma = nc.sync.dma_start

        keep = ctx.enter_context(tc.tile_pool(name="keep", bufs=1))
        stat = ctx.enter_context(tc.tile_pool(name="stat", bufs=8))
        x2_pool = ctx.enter_context(tc.tile_pool(name="x2p", bufs=1))
        eps_t = keep.tile([128, 1], f32, tag="eps")
        nc.vector.memset(eps_t[:, :], EPS)
        ident = keep.tile([128, 128], bf16, tag="ident")
        make_identity(nc, ident[:, :])
        ident_h = keep.tile([128, 128], f16, tag="identh")
        make_identity(nc, ident_h[:, :])
        ident_f = keep.tile([128, 128], f32, tag="identf")
        make_identity(nc, ident_f[:, :])


        # Persistent pools (never released) must sit below the transient
        # phase pools on the allocation stack.
        afw = ctx.enter_context(tc.tile_pool(name="afw", bufs=1))
        ai_pool = ctx.enter_context(tc.tile_pool(name="aip", bufs=1))
        mlpw = ctx.enter_context(tc.tile_pool(name="mlpw", bufs=1))

        # ---- DMA priority order: x first (gates LN1 -> DFT), then the DFT
        # factor matrices, then everything else (prefetch overlapping compute).
        x_pool = tc.alloc_tile_pool(name="xp", bufs=1)
        fw_pool = tc.alloc_tile_pool(name="fwp", bufs=1)
        xr = xd.ap().rearrange("b (n p) c -> n p b c", p=128)
        x_t = [x_pool.tile([128, B_LOC, DIM], bf16, tag=f"x{i}", name=f"x{i}")
               for i in range(8)]
        fcos_t = fw_pool.tile([128, 80], bf16, tag="fcos")
        fsin_t = fw_pool.tile([128, 80], bf16, tag="fsin")
        # first chunk + stage-1 factors first; odd chunks go out on the
        # scalar hwdge queue so both DMA rings pull concurrently.
        dma(out=x_t[0][:, :, :], in_=xr[0])
        dma(out=fcos_t[:, :], in_=fcosd.ap())
        dma(out=fsin_t[:, :], in_=fsind.ap())
        for i in range(1, 8):
            eng = dma if i % 2 == 0 else nc.scalar.dma_start
            eng(out=x_t[i][:, :, :], in_=xr[i])
        fhr_t, fhi_t = [], []
        for q in range(5):
            tr_ = fw_pool.tile([128, 256], bf16, tag=f"fhr{q}")
            ti_ = fw_pool.tile([128, 256], bf16, tag=f"fhi{q}")
            dma(out=tr_[:, :], in_=fhrd.ap()[q])
            dma(out=ti_[:, :], in_=fhid.ap()[q])
            fhr_t.append(tr_)
            fhi_t.append(ti_)

        w1ca = afw.tile([96, 768], bf16, tag="w1ca")
        w1cb = afw.tile([96, 768], bf16, tag="w1cb")
        w2c = afw.tile([97, 768], bf16, tag="w2c")
        b1c = afw.tile([96, 8], f32, tag="b1c")
        dcv = afw.tile([96, 8], f32, tag="dcv")
        dma(out=w1ca[:, :], in_=w1cad.ap())
        dma(out=w1cb[:, :], in_=w1cbd.ap())
        dma(out=w2c[:, :], in_=w2cd.ap())
        dma(out=b1c[:, :], in_=b1cd.ap())
        dma(out=dcv[:, :], in_=dcvd.ap())

        # Tiles for the later-phase weights (allocated now for pool-stack
        # order; their DMAs are issued after the DFT so they don't block the
        # DFT-internal shuffle DMAs on the queue).  hp memsets run on the
        # idle GpSimd engine during the startup DMA window.
        av = [ai_pool.tile([128, 1024], bf16, tag=f"av{q}", name=f"av{q}")
              for q in range(10)]
        w1fq = mlpw.tile([128, 3, HID], f8, tag="w1fq", name="w1fq")
        b1fv = mlpw.tile([128, 12], f32, tag="b1fv")
        w0v = mlpw.tile([128, 12], f32, tag="w0v")
        dwbv = mlpw.tile([128, 12], f32, tag="dwbv")
        dvev = mlpw.tile([128, 36], f32, tag="dvev")
        ddiag_t = [mlpw.tile([128, len(TAPS_PE) * 128], bf16, tag=f"dd{t}",
                             name=f"dd{t}") for t in range(12)]
        hp_t = []
        for t in range(12):
            hp = mlpw.tile([128, HP_LEN], bf16, tag=f"hp{t}")
            nc.gpsimd.memset(hp[:, :], 0.0)
            hp_t.append(hp)

        def layer_stats(xs, tagsuf):
            st = stat.tile([128, 6], f32, tag="st" + tagsuf)
            nc.vector.bn_stats(out=st[:, :], in_=xs)
            mv = stat.tile([128, 2], f32, tag="mv" + tagsuf)
            nc.vector.bn_aggr(out=mv[:, :], in_=st[:, :])
            rstd = stat.tile([128, 1], f32, tag="rs" + tagsuf)
            nc.scalar.activation(out=rstd[:, :], in_=mv[:, 1:2],
                                 func=ACTF.Sqrt, bias=eps_t[:, :])
            nc.vector.reciprocal(out=rstd[:, :], in_=rstd[:, :])
            return mv, rstd

        def layer_norm_act(out, xs, tagsuf, on_scalar=True):
            # normalize: out = xs*rstd + (-m*rstd); engine selectable so the
            # work can be split between ACT and DVE.
            mv, rstd = layer_stats(xs, tagsuf)
            if on_scalar:
                nmr = stat.tile([128, 1], f32, tag="nmr" + tagsuf)
                nc.vector.tensor_scalar(out=nmr[:, :], in0=mv[:, 0:1],
                                        scalar1=rstd[:, :], scalar2=-1.0,
                                        op0=ALU.mult, op1=ALU.mult)
                nc.scalar.activation(out=out, in_=xs, func=ACTF.Identity,
                                     scale=rstd[:, :], bias=nmr[:, :])
            else:
                nc.vector.tensor_scalar(out=out, in0=xs,
                                        scalar1=mv[:, 0:1],
                                        scalar2=rstd[:, :],
                                        op0=ALU.subtract, op1=ALU.mult)

        # ---------------- Phase A: LN1 (b-major) ----------------
        z1_pool = tc.alloc_tile_pool(name="z1p", bufs=1)
        n1_pool = tc.alloc_tile_pool(name="n1p", bufs=1)

        if not ln1_trivial:
            g_bc = n1_pool.tile([128, DIM], bf16, tag="g_bc")
            b_bc = n1_pool.tile([128, DIM], bf16, tag="b_bc")
            dma(out=g_bc[:, :], in_=ln1gd.ap())
            dma(out=b_bc[:, :], in_=ln1bd.ap())

        z1_t, n1_t = [], []
        with nc.named_scope("ln1"):
            for i in range(8):
                z1 = z1_pool.tile([128, B_LOC, DIM], bf16, tag=f"z1_{i}")
                z1_t.append(z1)
                if ln1_trivial:
                    n1_t.append(z1)  # n1 == z1 when gamma=1, beta=0
                else:
                    n1_t.append(n1_pool.tile([128, B_LOC, DIM], bf16,
                                             tag=f"n1_{i}"))
            for b in range(B_LOC):
                for i in range(8):
                    xs = x_t[i][:, b, :]
                    layer_norm_act(z1_t[i][:, b, :], xs, "1",
                                   on_scalar=(i % 2 == 0))
                    if not ln1_trivial:
                        nc.vector.tensor_mul(out=n1_t[i][:, b, :],
                                             in0=z1_t[i][:, b, :],
                                             in1=g_bc[:, :])
                        nc.vector.tensor_add(out=n1_t[i][:, b, :],
                                             in0=n1_t[i][:, b, :],
                                             in1=b_bc[:, :])

        # ---------------- Phase B: separable forward DFT ----------------
        xf_pool = tc.alloc_tile_pool(name="xfp", bufs=1)
        xf_t = [xf_pool.tile([96, 2 * NMODE], bf16, tag=f"xf{mt}",
                             name=f"xf{mt}") for mt in range(8)]
        with nc.named_scope("dft"):
            with tc.tile_pool(name="xwp", bufs=4) as xw_pool, \
                 tc.tile_pool(name="xhp", bufs=1) as xh_pool, \
                 tc.tile_pool(name="s1ps", bufs=4, space="PSUM") as s1_ps, \
                 tc.tile_pool(name="s2ps", bufs=4, space="PSUM") as s2_ps:
                # xh: [128 (h, kw'), 5 (q), B, DIM]
                xh_r = xh_pool.tile([128, 5, B_LOC, DIM], bf16, tag="xhr")
                xh_i = xh_pool.tile([128, 5, B_LOC, DIM], bf16, tag="xhi")
                for b in range(B_LOC):
                    # stage 1: W-direction rDFT per position chunk
                    for i in range(8):
                        psr = s1_ps.tile([80, DIM], f32, tag="s1ps")
                        psi = s1_ps.tile([80, DIM], f32, tag="s1ps")
                        nc.tensor.matmul(psr[:, :], fcos_t[:, :],
                                         z1_t[i][:, b, :],
                                         start=True, stop=True)
                        nc.tensor.matmul(psi[:, :], fsin_t[:, :],
                                         z1_t[i][:, b, :],
                                         start=True, stop=True)
                        xwr = xw_pool.tile([80, DIM], bf16, tag="xwr")
                        xwi = xw_pool.tile([80, DIM], bf16, tag="xwi")
                        nc.scalar.copy(out=xwr[:, :], in_=psr[:, :])
                        nc.vector.tensor_copy(out=xwi[:, :], in_=psi[:, :])
                        # (h4, kw', q) partitions -> (h, kw') x (q) free
                        dma(out=xh_r[16 * i: 16 * (i + 1), :, b, :],
                            in_=xwr[:, :])
                        dma(out=xh_i[16 * i: 16 * (i + 1), :, b, :],
                            in_=xwi[:, :])
                    # stage 2: H-direction DFT; lhsT = data, rhs = factors
                    for q in range(5):
                        ncol = 128 if q < 4 else 32
                        for c96 in range(4):
                            ps2 = s2_ps.tile([96, 256], f32, tag="s2ps")
                            nc.tensor.matmul(
                                ps2[:, : 2 * ncol],
                                xh_r[:, q, b, 96 * c96: 96 * (c96 + 1)],
                                fhr_t[q][:, : 2 * ncol],
                                start=True, stop=False)
                            nc.tensor.matmul(
                                ps2[:, : 2 * ncol],
                                xh_i[:, q, b, 96 * c96: 96 * (c96 + 1)],
                                fhi_t[q][:, : 2 * ncol],
                                start=False, stop=True)
                            xf2d = xf_t[b * 4 + c96][:, :]
                            dest = bass.AP(
                                tensor=xf2d.tensor,
                                offset=xf2d.offset + 128 * q,
                                ap=[xf2d.ap[0], [NMODE, 2], [1, ncol]])
                            cp_eng = (nc.scalar.copy if c96 % 2 == 0
                                      else nc.vector.tensor_copy)
                            cp_eng(
                                out=dest,
                                in_=ps2[:, : 2 * ncol]
                                    .rearrange("p (r m) -> p r m", r=2))

        # ---- weight prefetch: issued here so these queue BEHIND the DFT
        # shuffle DMAs but still land well before the idft / MLP need them.
        aoff = 0
        for q in range(10):
            rows = IDFT_CHUNKS[q // 2]
            dma(out=av[q][:rows, :], in_=aid.ap()[aoff: aoff + rows, :])
            aoff += rows
        dma(out=w1fq[:, :, :], in_=w1fd.ap())
        dma(out=b1fv[:, :], in_=b1fvd.ap())
        dma(out=w0v[:, :], in_=w0vd.ap())
        dma(out=dwbv[:, :], in_=dwbvd.ap())
        dma(out=dvev[:, :], in_=dvevd.ap())
        for t in range(12):
            dma(out=ddiag_t[t][:, :], in_=ddiagd.ap()[t])

        # ---------------- Phase C: frequency-domain block MLP ----------------
        o2s_pool = tc.alloc_tile_pool(name="o2sp", bufs=1)
        o2s_t = [o2s_pool.tile([IDFT_CHUNKS[t], B_LOC, DIM, 2], bf16,
                               tag=f"o2s{t}", name=f"o2s{t}") for t in range(5)]
        with nc.named_scope("blockmlp"):
            with tc.tile_pool(name="o1p", bufs=8) as o1_pool, \
                 tc.tile_pool(name="l1ps", bufs=3, space="PSUM") as l1_ps, \
                 tc.tile_pool(name="l2ps", bufs=2, space="PSUM") as l2_ps:
                for b in range(B_LOC):
                    o1_b = []
                    for n in range(NUM_BLOCKS):
                        xf = xf_t[b * 4 + n // 2]
                        o1 = o1_pool.tile([97, NMODE], bf16, tag="o1")
                        nc.gpsimd.memset(o1[96:97, :], 1.0)
                        ps = l1_ps.tile([96, 544], f32, tag="l1ps")
                        for w0, wlen in ((0, 512), (512, 32)):
                            nc.tensor.matmul(
                                ps[:, w0: w0 + wlen],
                                w1ca[:, 96 * n: 96 * (n + 1)],
                                xf[:, w0: w0 + wlen],
                                start=True, stop=False)
                            nc.tensor.matmul(
                                ps[:, w0: w0 + wlen],
                                w1cb[:, 96 * n: 96 * (n + 1)],
                                xf[:, NMODE + w0: NMODE + w0 + wlen],
                                start=False, stop=True)
                        if not ln1_trivial:
                            # DC-mode correction from LN1 beta (zero if trivial)
                            nc.vector.tensor_add(out=ps[:, 0:1], in0=ps[:, 0:1],
                                                 in1=dcv[:, n: n + 1])
                        nc.scalar.activation(out=o1[0:96, :], in_=ps[:, :],
                                             func=ACTF.Relu,
                                             bias=b1c[:, n: n + 1])
                        o1_b.append(o1)
                    for mc in range(5):
                        rows = IDFT_CHUNKS[mc]
                        for hn in range(2):
                            ps2 = l2_ps.tile([128, 384], f32, tag="l2ps")
                            for j in range(4):
                                n = hn * 4 + j
                                nc.tensor.matmul(
                                    ps2[:rows, 96 * j: 96 * (j + 1)],
                                    o1_b[n][:, 128 * mc: 128 * mc + rows],
                                    w2c[:, 96 * n: 96 * (n + 1)],
                                    start=True, stop=True)
                            cp2 = (nc.scalar.copy if (mc + hn) % 2 == 0
                                   else nc.vector.tensor_copy)
                            cp2(
                                out=o2s_t[mc][:rows, b,
                                              hn * 192: (hn + 1) * 192, :]
                                    .rearrange("p c r -> p (c r)"),
                                in_=ps2[:rows, :])

        # softshrink in place
        with nc.named_scope("softshrink"):
            with tc.tile_pool(name="sshp", bufs=2) as ssh_pool:
                for t in range(5):
                    rows = IDFT_CHUNKS[t]
                    flat = o2s_t[t][:, :, :, :].rearrange("p b c r -> p (b c r)")
                    tmp = ssh_pool.tile([128, B_LOC * DIM * 2], bf16, tag="ssh")
                    nc.vector.tensor_scalar(out=tmp[:rows, :], in0=flat,
                                            scalar1=-LAM, scalar2=LAM,
                                            op0=ALU.max, op1=ALU.min)
                    nc.vector.tensor_tensor(
                        out=flat, in0=flat, in1=tmp[:rows, :],
                        op=ALU.subtract)

        # ---------------- Phase D: inverse DFT + residual ----------------
        x2_t = []
        with nc.named_scope("idft"):
            with tc.tile_pool(name="idftps", bufs=4, space="PSUM") as idft_ps:
                for mt in range(8):
                    x2 = x2_pool.tile([128, B_LOC, DIM], bf16, tag=f"x2_{mt}")
                    for b in range(B_LOC):
                        ps = idft_ps.tile([128, DIM], f32, tag="idftps")
                        for q in range(10):
                            rows = IDFT_CHUNKS[q // 2]
                            nc.tensor.matmul(
                                ps[:, :],
                                av[q][:rows, 128 * mt: 128 * (mt + 1)],
                                o2s_t[q // 2][:rows, b, :, q % 2],
                                start=(q == 0), stop=False)
                        # + n1 residual (z1 when LN1 is trivial)
                        nc.tensor.matmul(
                            ps[:, :], ident[:, :], n1_t[mt][:, b, :],
                            start=False, stop=True)
                        nc.vector.tensor_add(out=x2[:, b, :],
                                             in0=x_t[mt][:, b, :], in1=ps[:, :])
                    x2_t.append(x2)
        o2s_pool.release()
        xf_pool.release()
        n1_pool.release()
        z1_pool.release()
        fw_pool.release()
        x_pool.release()

        # ---------------- Phase E: MLP (late weights) ----------------
        mlpw2 = ctx.enter_context(tc.tile_pool(name="mlpw2", bufs=1))
        w2fq = mlpw2.tile([128, 12, DIM], f8, tag="w2fq", name="w2fq")
        dma(out=w2fq[:, :, :], in_=w2fd.ap())
        b2f = mlpw2.tile([1, DIM], bf16, tag="b2f")
        dma(out=b2f[:, :], in_=b2fd.ap())
        ones_b = mlpw2.tile([1, 1024], bf16, tag="ones")
        nc.vector.memset(ones_b[:, :], 1.0)
        yq = mlpw2.tile([128, 12, 1024], f8, tag="yq", name="yq")
        # z2T as one [128, 3 (ch-chunk), 1024 (pos)] tile
        z2T = mlpw2.tile([128, 3, 1024], f8, tag="z2T", name="z2T")

        with tc.tile_pool(name="z2p", bufs=3) as z2_pool, \
             tc.tile_pool(name="accp", bufs=3) as acc_pool, \
             tc.tile_pool(name="outp", bufs=3) as out_pool, \
             tc.tile_pool(name="mlpps", bufs=2, space="PSUM") as mlp_ps, \
             tc.tile_pool(name="cvps", bufs=2, space="PSUM") as cv_ps:
            for b in range(B_LOC):
                with nc.named_scope("ln2t"):
                    for i in range(8):
                        xs = x2_t[i][:, b, :]
                        z2 = z2_pool.tile([128, DIM], f16, tag="z2")
                        layer_norm_act(z2[:, :], xs, "2",
                                       on_scalar=(i % 2 == 0))
                        pstf = mlp_ps.tile([128, 512], f32, tag="fc1ps")
                        pst = pstf[:, :].bitcast(f16)
                        for c in range(3):
                            nc.tensor.transpose(
                                pst[:, 128 * c: 128 * (c + 1)],
                                z2[:, 128 * c: 128 * (c + 1)],
                                ident_h[:, :])
                        nc.scalar.copy(
                            out=z2T[:, :, 128 * i: 128 * (i + 1)],
                            in_=pst[:, :384].rearrange("p (c m) -> p c m", c=3))
                with nc.named_scope("fc1"):
                    for t in range(12):
                        for ncb in range(2):
                            ps = mlp_ps.tile([128, 512], f32, tag="fc1ps")
                            nc.tensor.matmul(
                                ps[:, :],
                                w1fq[:, 0:2, 128 * t: 128 * (t + 1)],
                                z2T[:, 0:2, 512 * ncb: 512 * (ncb + 1)],
                                start=True, stop=False, perf_mode=DR)
                            nc.tensor.matmul(
                                ps[:, :],
                                w1fq[:, 2, 128 * t: 128 * (t + 1)],
                                z2T[:, 2, 512 * ncb: 512 * (ncb + 1)],
                                start=False, stop=True)
                            hp2d = hp_t[t][:, :]
                            dest = bass.AP(
                                tensor=hp2d.tensor,
                                offset=hp2d.offset + 36 + 544 * ncb,
                                ap=[hp2d.ap[0], [34, 16], [1, 32]])
                            nc.scalar.activation(
                                out=dest,
                                in_=ps[:, :].rearrange("p (h w) -> p h w", w=32),
                                func=ACTF.Identity, bias=b1fv[:, t: t + 1],
                                scale=1.0 / FP8_WSCALE)
                with nc.named_scope("conv"):
                    for t in range(12):
                        hp2d = hp_t[t][:, :]
                        # fp16 SBUF accumulator: center + 3 taps on DVE.
                        # Each tap is tensor_scalar (4x mode) + tensor_tensor
                        # (2x mode) -- cheaper than 1x scalar_tensor_tensor.
                        acc = acc_pool.tile([128, CONV_LEN], f16, tag="acc")
                        nc.vector.tensor_scalar(
                            out=acc[:, :], in0=shifted(hp2d, 35, CONV_LEN),
                            scalar1=w0v[:, t: t + 1],
                            scalar2=dwbv[:, t: t + 1],
                            op0=ALU.mult, op1=ALU.add)
                        for j, (dy, dx) in enumerate(TAPS_DVE):
                            dd = dy * PADW + dx
                            tmp = acc_pool.tile([128, CONV_LEN], bf16,
                                                tag="tmpt")
                            nc.vector.tensor_scalar(
                                out=tmp[:, :],
                                in0=shifted(hp2d, 35 + dd, CONV_LEN),
                                scalar1=dvev[:, 12 * j + t: 12 * j + t + 1],
                                scalar2=None,
                                op0=ALU.mult)
                            nc.vector.tensor_tensor(
                                out=acc[:, :], in0=tmp[:, :], in1=acc[:, :],
                                op=ALU.add)
                        # PE taps: one 3-bank PSUM tile; tap-outer order so
                        # consecutive matmuls reuse the loaded weights.
                        cps = cv_ps.tile([128, CONV_LEN], f32, tag="cvps")
                        for j, (dy, dx) in enumerate(TAPS_PE):
                            dd = dy * PADW + dx
                            for w0, wlen in ((0, 512), (512, 512), (1024, 64)):
                                nc.tensor.matmul(
                                    cps[:, w0: w0 + wlen],
                                    ddiag_t[t][:, 128 * j: 128 * (j + 1)],
                                    shifted(hp2d, 35 + w0 + dd, wlen),
                                    start=(j == 0), stop=False,
                                    skip_group_check=True)
                        # inject the DVE accumulator via identity matmul
                        for w0, wlen in ((0, 512), (512, 512), (1024, 64)):
                            nc.tensor.matmul(
                                cps[:, w0: w0 + wlen],
                                ident_h[:, :],
                                shifted(acc[:, :], w0, wlen),
                                start=False, stop=True,
                                skip_group_check=True)
                        cps2d = cps[:, :]
                        for hf in range(2):
                            nc.scalar.activation(
                                out=yq[:, t, 512 * hf: 512 * (hf + 1)]
                                    .rearrange("p (h w) -> p h w", w=32),
                                in_=bass.AP(tensor=cps2d.tensor,
                                            offset=cps2d.offset + 1
                                            + CONV_HALF * hf,
                                            ap=[cps2d.ap[0], [34, 16], [1, 32]]),
                                func=ACTF.Gelu)
                with nc.named_scope("fc2"):
                    for mc in range(8):
                        psf = mlp_ps.tile([128, 512], f32, tag="fc1ps")
                        ps = psf[:, :DIM]
                        for j in range(6):
                            nc.tensor.matmul(
                                ps[:, :],
                                yq[:, 2 * j: 2 * j + 2, 128 * mc: 128 * (mc + 1)],
                                w2fq[:, 2 * j: 2 * j + 2, :],
                                start=(j == 0), stop=False, perf_mode=DR)
                        nc.tensor.matmul(
                            ps[:, :], ones_b[:, 128 * mc: 128 * (mc + 1)],
                            b2f[:, :], start=False, stop=True)
                        # 1/16 weight-scale compensation + x2 residual, on DVE
                        ot = out_pool.tile([128, DIM], bf16, tag="out")
                        nc.vector.scalar_tensor_tensor(
                            out=ot[:, :], in0=ps[:, :], scalar=1.0 / FP8_WSCALE,
                            in1=x2_t[mc][:, b, :], op0=ALU.mult, op1=ALU.add)
                        dma(out=yd.ap()[b, 128 * mc: 128 * (mc + 1), :],
                            in_=ot[:, :])

    return nc


_NC_CACHE = {}


def kernel(**inputs):
    from concourse.bass_utils import run_bass_kernel_spmd

    x = np.ascontiguousarray(np.asarray(inputs["x"], np.float32))
    assert int(inputs["H"]) == H and int(inputs["W"]) == W
    der = host_derived(inputs)
    trivial = (np.allclose(np.asarray(inputs["ln1_g"]), 1.0)
               and np.allclose(np.asarray(inputs["ln1_b"]), 0.0))

    if trivial not in _NC_CACHE:
        nc = build_nc(ln1_trivial=trivial)
        nc.compile()
        _NC_CACHE[trivial] = nc
    nc = _NC_CACHE[trivial]

    x_bf = x.astype(ml_dtypes.bfloat16)
    in_maps = []
    for c in range(N_CORES):
        m = dict(der)
        m["x"] = np.ascontiguousarray(x_bf[c * B_LOC: (c + 1) * B_LOC])
        in_maps.append(m)
    res = run_bass_kernel_spmd(nc, in_maps, core_ids=list(range(N_CORES)))
    out = np.concatenate([res.results[c]["y"] for c in range(N_CORES)], axis=0)
    return out.astype(np.float32)

